# revision 3
# baseline (speedup 1.0000x reference)
"""2-layer AnalogLSTM (B=32, T=256, IN=512, H=1024) on 8 TRN2 NeuronCores.

Sharding: tensor-parallel over the 4H gate dimension. Core k owns h-dims
[k*128,(k+1)*128) of both layers: it holds the 4x128 = 512 gate rows
(i, f, o, g blocks) needed to update its h/c slice. Per step, each core
computes its transposed gate tile gates.T (4 PSUM banks of [128, B]) with
weight-stationary matmuls (lhsT = W.T chunk [128,128], rhs = h.T chunk
[128,B] bf16), applies the LSTM cell elementwise on [128,B] tiles, then
AllGathers the new h.T slice so every core has the full h.T [8,128,B]
for the next step's contraction. Layer 1 is emitted one step behind
layer 0 so each layer's AllGather latency hides under the other layer's
matmuls; comm instructions are emitted on the sync ring in the order
their wait conditions clear.
"""

import sys

if "/opt/trn_rl_repo" not in sys.path:
    sys.path.insert(0, "/opt/trn_rl_repo")

import numpy as np
import ml_dtypes

import concourse.bacc as bacc
import concourse.mybir as mybir
import concourse.tile as tile
from concourse.bass_utils import run_bass_kernel_spmd

NC_CORES = 8
B = 32
H = 1024
IN = 512
KC_IN = IN // 128  # 4 contraction chunks for x
KC_H = H // 128    # 8 contraction chunks for h
NM = 4             # gate chunks per core: i, f, o, g
F32 = mybir.dt.float32
BF16 = mybir.dt.bfloat16
SIG = mybir.ActivationFunctionType.Sigmoid
TANH = mybir.ActivationFunctionType.Tanh
# gate block order in the reference weights: [i, f, g, o]; our m order: i, f, o, g
GATE_BLOCKS = (0, 1, 3, 2)

_NC_CACHE = {}


def build_nc(T: int):
    if T in _NC_CACHE:
        return _NC_CACHE[T]
    nc = bacc.Bacc("TRN2", target_bir_lowering=False, debug=False,
                   num_devices=NC_CORES)

    xT = nc.dram_tensor("xT", [T, 128, KC_IN * B], BF16, kind="ExternalInput")
    w0x = nc.dram_tensor("w0x", [128, KC_IN * NM * 128], BF16, kind="ExternalInput")
    w0h = nc.dram_tensor("w0h", [128, KC_H * NM * 128], BF16, kind="ExternalInput")
    w1x = nc.dram_tensor("w1x", [128, KC_H * NM * 128], BF16, kind="ExternalInput")
    w1h = nc.dram_tensor("w1h", [128, KC_H * NM * 128], BF16, kind="ExternalInput")
    bias0 = nc.dram_tensor("bias0", [128, NM], F32, kind="ExternalInput")
    bias1 = nc.dram_tensor("bias1", [128, NM], F32, kind="ExternalInput")
    y = nc.dram_tensor("y", [T, 128, B], BF16, kind="ExternalOutput")
    hf = nc.dram_tensor("hf", [2, 128, B], F32, kind="ExternalOutput")
    cf = nc.dram_tensor("cf", [2, 128, B], F32, kind="ExternalOutput")

    rg = [list(range(NC_CORES))]

    with tile.TileContext(nc) as tc:
        with (
            tc.tile_pool(name="wp", bufs=1) as wp,
            tc.tile_pool(name="sb", bufs=2) as sb,
            tc.tile_pool(name="xp", bufs=4) as xp,
            tc.tile_pool(name="pp", bufs=1, space="PSUM") as pp,
            tc.tile_pool(name="dr", bufs=2, space="DRAM") as dr,
        ):
            # ---- load weights/biases once ----
            w0x_sb = wp.tile([128, KC_IN * NM * 128], BF16, name="w0x_sb")
            w0h_sb = wp.tile([128, KC_H * NM * 128], BF16, name="w0h_sb")
            w1x_sb = wp.tile([128, KC_H * NM * 128], BF16, name="w1x_sb")
            w1h_sb = wp.tile([128, KC_H * NM * 128], BF16, name="w1h_sb")
            b0_sb = wp.tile([128, NM], F32, name="b0_sb")
            b1_sb = wp.tile([128, NM], F32, name="b1_sb")
            nc.gpsimd.dma_start(w0x_sb[:], w0x[:])
            nc.gpsimd.dma_start(w0h_sb[:], w0h[:])
            nc.gpsimd.dma_start(w1x_sb[:], w1x[:])
            nc.gpsimd.dma_start(w1h_sb[:], w1h[:])
            nc.gpsimd.dma_start(b0_sb[:], bias0[:])
            nc.gpsimd.dma_start(b1_sb[:], bias1[:])
            bias_sb = [b0_sb, b1_sb]

            # ---- initial state ----
            c_prev = [None, None]
            for L in range(2):
                ct = sb.tile([128, B], F32, tag=f"c{L}", name=f"c{L}_init")
                nc.vector.memset(ct[:], 0.0)
                c_prev[L] = ct
            hg_prev = [None, None]  # gathered h.T [128, KC_H, B] per layer

            def wtile(w_sb, c, m):
                return w_sb[:, (c * NM + m) * 128:(c * NM + m + 1) * 128]

            def mm_group(G, parts, t):
                """parts: list of (w_sb, rhs_fn, kc) accumulated into PSUM banks.

                G is a list of NM single-bank PSUM tiles [128, B] (one
                accumulation group per bank — PSUM zero regions are
                bank-granular). m-inner so chunks stream in order.
                """
                total = sum(kc for (_, _, kc) in parts)
                n = 0
                for w_sb, rhs_fn, kc in parts:
                    for c in range(kc):
                        for m in range(NM):
                            nc.tensor.matmul(
                                G[m][:],
                                lhsT=wtile(w_sb, c, m),
                                rhs=rhs_fn(c),
                                start=(n == 0),
                                stop=(n == total - 1),
                            )
                        n += 1

            def chain(L, t, G):
                """LSTM cell elementwise on gates.T PSUM tile -> h.T bf16 tile."""
                bs = bias_sb[L]
                sf = sb.tile([128, B], F32, tag=f"sf{L}", name=f"sf{L}_{t}")
                si = sb.tile([128, B], F32, tag=f"si{L}", name=f"si{L}_{t}")
                so = sb.tile([128, B], F32, tag=f"so{L}", name=f"so{L}_{t}")
                tg = sb.tile([128, B], F32, tag=f"tg{L}", name=f"tg{L}_{t}")
                # m order: 0=i 1=f 2=o 3=g; f first so the DVE can start c*f early
                nc.scalar.activation(sf[:], G[:, B:2 * B], SIG, bias=bs[:, 1:2])
                nc.scalar.activation(si[:], G[:, 0:B], SIG, bias=bs[:, 0:1])
                nc.scalar.activation(tg[:], G[:, 3 * B:4 * B], TANH, bias=bs[:, 3:4])
                nc.scalar.activation(so[:], G[:, 2 * B:3 * B], SIG, bias=bs[:, 2:3])
                t1 = sb.tile([128, B], F32, tag=f"t1{L}", name=f"t1{L}_{t}")
                t2 = sb.tile([128, B], F32, tag=f"t2{L}", name=f"t2{L}_{t}")
                cn = sb.tile([128, B], F32, tag=f"c{L}", name=f"c{L}_{t}")
                nc.vector.tensor_mul(t1[:], sf[:], c_prev[L][:])
                nc.vector.tensor_mul(t2[:], si[:], tg[:])
                nc.vector.tensor_add(cn[:], t1[:], t2[:])
                c_prev[L] = cn
                tch = sb.tile([128, B], F32, tag=f"tc{L}", name=f"tc{L}_{t}")
                nc.scalar.activation(tch[:], cn[:], TANH)
                hb = sb.tile([128, B], BF16, tag=f"h{L}", name=f"h{L}_{t}")
                nc.vector.tensor_mul(hb[:], so[:], tch[:])
                return hb

            def ag_send(L, t, hb):
                """h.T slice -> bounce -> AllGather trigger (all on sync)."""
                bounce = dr.tile([128, B], BF16, tag=f"bo{L}", name=f"bo{L}_{t}")
                nc.sync.dma_start(bounce[:], hb[:])
                gath = dr.tile([NC_CORES * 128, B], BF16, addr_space="Shared",
                               tag=f"ga{L}", name=f"ga{L}_{t}", bufs=3)
                nc.sync.collective_compute(
                    "AllGather", mybir.AluOpType.bypass, replica_groups=rg,
                    ins=[bounce[:].opt()], outs=[gath[:].opt()],
                )
                return gath

            def ag_recv(L, t, gath):
                """Gathered DRAM [8*128, B] -> SBUF [128, KC_H, B] (2 halves)."""
                half = KC_H // 2
                hga = sb.tile([128, half, B], BF16, tag=f"hga{L}", name=f"hga{L}_{t}")
                hgb = sb.tile([128, half, B], BF16, tag=f"hgb{L}", name=f"hgb{L}_{t}")
                g3 = gath.rearrange("(c p) b -> p c b", p=128)
                nc.sync.dma_start(hga[:], g3[:, :half, :])
                nc.sync.dma_start(hgb[:], g3[:, half:, :])
                hg_prev[L] = (hga, hgb)

            def layer1_mms(tm):
                G1 = pp.tile([128, NM * B], F32, tag="G1", name=f"G1_{tm}")
                h0g = hg_prev[0]
                parts = [(w1x_sb, lambda c, _h=h0g: _h[c // 4][:, c % 4, :], KC_H)]
                if tm > 0:
                    h1g = hg_prev[1]
                    parts.append((w1h_sb, lambda c, _h=h1g: _h[c // 4][:, c % 4, :], KC_H))
                mm_group(G1, parts, tm)
                return G1

            # Emission order per block t (engines in brackets):
            #   xt(t) [gpsimd]
            #   L0x(t), L0h(t) [PE]; chain0(t) [ACT/DVE]
            #   bo0(t), TRIG0(t) [sync]
            #   hg1(t-2) [sync]      <- AG1(t-2) result, consumed by L1h(t-1)
            #   L1x(t-1), L1h(t-1) [PE]; chain1(t-1); y(t-1) [gpsimd]
            #   bo1(t-1), TRIG1(t-1) [sync]
            #   hg0(t) [sync]        <- AG0(t) result, consumed by L0h(t+1)/L1x(t)
            h1b = None
            last_h0 = None
            ga0 = None
            ga1_pend = {}  # t -> gath handle for AG1(t)
            for t in range(T):
                xt_tile = xp.tile([128, KC_IN * B], BF16, tag="xt", name=f"xt_{t}")
                nc.gpsimd.dma_start(xt_tile[:], xT[t])
                G0 = pp.tile([128, NM * B], F32, tag="G0", name=f"G0_{t}")
                parts = [(w0x_sb, lambda c, _x=xt_tile: _x[:, c * B:(c + 1) * B],
                          KC_IN)]
                if t > 0:
                    h0g = hg_prev[0]
                    parts.append((w0h_sb, lambda c, _h=h0g: _h[c // 4][:, c % 4, :], KC_H))
                mm_group(G0, parts, t)
                h0b = chain(0, t, G0)
                last_h0 = h0b
                ga0 = ag_send(0, t, h0b)

                if t - 2 in ga1_pend:
                    ag_recv(1, t - 2, ga1_pend.pop(t - 2))

                if t > 0:
                    tm = t - 1
                    G1 = layer1_mms(tm)
                    h1b = chain(1, tm, G1)
                    nc.gpsimd.dma_start(y[tm], h1b[:])
                    if tm < T - 1:
                        ga1_pend[tm] = ag_send(1, tm, h1b)

                ag_recv(0, t, ga0)

            # ---------- layer 1, final step ----------
            tm = T - 1
            G1 = layer1_mms(tm)
            h1b = chain(1, tm, G1)
            nc.gpsimd.dma_start(y[tm], h1b[:])

            # ---------- final h/c ----------
            h0f = sb.tile([128, B], F32, tag="h0f", name="h0f")
            h1f = sb.tile([128, B], F32, tag="h1f", name="h1f")
            nc.vector.tensor_copy(h0f[:], last_h0[:])
            nc.vector.tensor_copy(h1f[:], h1b[:])
            nc.sync.dma_start(hf[0], h0f[:])
            nc.sync.dma_start(hf[1], h1f[:])
            nc.sync.dma_start(cf[0], c_prev[0][:])
            nc.sync.dma_start(cf[1], c_prev[1][:])

    nc.compile()
    _NC_CACHE[T] = nc
    return nc


def _pack_w(W_eff, k):
    """W_eff [4H, D] -> per-core lhsT pack [128, (D/128)*NM*128] bf16.

    pack[p, (c*NM+m)*128 + col] = W_eff[GATE_BLOCKS[m]*H + k*128 + col, c*128 + p]
    """
    D = W_eff.shape[1]
    kc = D // 128
    rows = np.concatenate(
        [bm * H + k * 128 + np.arange(128) for bm in GATE_BLOCKS])
    arr = W_eff[rows].reshape(NM, 128, kc, 128)      # [m, col, c, p]
    arr = arr.transpose(3, 2, 0, 1).reshape(128, kc * NM * 128)
    return np.ascontiguousarray(arr.astype(ml_dtypes.bfloat16))


def _pack_bias(b_eff, k):
    cols = [b_eff[bm * H + k * 128:bm * H + k * 128 + 128] for bm in GATE_BLOCKS]
    return np.ascontiguousarray(np.stack(cols, axis=1).astype(np.float32))


def prep_in_maps(x, w_ih0, b_ih0, a_ih0, w_hh0, b_hh0, a_hh0,
                 w_ih1, b_ih1, a_ih1, w_hh1, b_hh1, a_hh1):
    x = np.asarray(x, dtype=np.float32)
    Bx, T, INx = x.shape
    assert (Bx, INx) == (B, IN)
    W0x = np.asarray(w_ih0, np.float32) * np.float32(a_ih0)
    W0h = np.asarray(w_hh0, np.float32) * np.float32(a_hh0)
    W1x = np.asarray(w_ih1, np.float32) * np.float32(a_ih1)
    W1h = np.asarray(w_hh1, np.float32) * np.float32(a_hh1)
    bias0 = np.asarray(b_ih0, np.float32) + np.asarray(b_hh0, np.float32)
    bias1 = np.asarray(b_ih1, np.float32) + np.asarray(b_hh1, np.float32)

    # xT[t, p, c*B + b] = x[b, t, c*128 + p]
    xT = x.reshape(B, T, KC_IN, 128).transpose(1, 3, 2, 0).reshape(
        T, 128, KC_IN * B)
    xT = np.ascontiguousarray(xT.astype(ml_dtypes.bfloat16))

    in_maps = []
    for k in range(NC_CORES):
        in_maps.append({
            "xT": xT,
            "w0x": _pack_w(W0x, k),
            "w0h": _pack_w(W0h, k),
            "w1x": _pack_w(W1x, k),
            "w1h": _pack_w(W1h, k),
            "bias0": _pack_bias(bias0, k),
            "bias1": _pack_bias(bias1, k),
        })
    return in_maps, T


def assemble_outputs(results, T):
    ys = np.stack([np.asarray(r["y"], dtype=np.float32) for r in results], axis=1)
    out = ys.transpose(3, 0, 1, 2).reshape(B, T, H)
    hfs = np.stack([np.asarray(r["hf"], dtype=np.float32) for r in results], axis=1)
    h_final = hfs.transpose(0, 3, 1, 2).reshape(2, B, H)
    cfs = np.stack([np.asarray(r["cf"], dtype=np.float32) for r in results], axis=1)
    c_final = cfs.transpose(0, 3, 1, 2).reshape(2, B, H)
    return out, h_final, c_final


def kernel(x, w_ih0, b_ih0, a_ih0, w_hh0, b_hh0, a_hh0,
           w_ih1, b_ih1, a_ih1, w_hh1, b_hh1, a_hh1):
    in_maps, T = prep_in_maps(
        x, w_ih0, b_ih0, a_ih0, w_hh0, b_hh0, a_hh0,
        w_ih1, b_ih1, a_ih1, w_hh1, b_hh1, a_hh1)
    nc = build_nc(T)
    res = run_bass_kernel_spmd(nc, in_maps, list(range(NC_CORES)))
    out, h_final, c_final = assemble_outputs(res.results, T)
    return (np.asarray(out, np.float32), np.asarray(h_final, np.float32),
            np.asarray(c_final, np.float32))


# revision 4
# speedup vs baseline: 1.2394x; 1.2394x over previous
"""2-layer AnalogLSTM (B=32, T=256, IN=512, H=1024) on 8 TRN2 NeuronCores.

Sharding: tensor-parallel over the 4H gate dimension. Core k owns h-dims
[k*128,(k+1)*128) of both layers: it holds the 4x128 = 512 gate rows
(i, f, o, g blocks) needed to update its h/c slice. Per step, each core
computes its transposed gate tile gates.T (4 PSUM banks of [128, B]) with
weight-stationary matmuls (lhsT = W.T chunk [128,128], rhs = h.T chunk
[128,B] bf16), applies the LSTM cell elementwise on [128,B] tiles, then
AllGathers the new h.T slice so every core has the full h.T [8,128,B]
for the next step's contraction. Layer 1 is emitted one step behind
layer 0 so each layer's AllGather latency hides under the other layer's
matmuls; comm instructions are emitted on the sync ring in the order
their wait conditions clear.
"""

import sys

if "/opt/trn_rl_repo" not in sys.path:
    sys.path.insert(0, "/opt/trn_rl_repo")

import numpy as np
import ml_dtypes

import concourse.bacc as bacc
import concourse.mybir as mybir
import concourse.tile as tile
from concourse.bass_utils import run_bass_kernel_spmd

NC_CORES = 8
B = 32
H = 1024
IN = 512
KC_IN = IN // 128  # 4 contraction chunks for x
KC_H = H // 128    # 8 contraction chunks for h
NM = 4             # gate chunks per core: i, f, o, g
F32 = mybir.dt.float32
BF16 = mybir.dt.bfloat16
SIG = mybir.ActivationFunctionType.Sigmoid
TANH = mybir.ActivationFunctionType.Tanh
# gate block order in the reference weights: [i, f, g, o]; our m order: i, f, o, g
GATE_BLOCKS = (0, 1, 3, 2)

_NC_CACHE = {}


def build_nc(T: int):
    if T in _NC_CACHE:
        return _NC_CACHE[T]
    nc = bacc.Bacc("TRN2", target_bir_lowering=False, debug=False,
                   num_devices=NC_CORES)

    xT = nc.dram_tensor("xT", [T, 128, KC_IN * B], BF16, kind="ExternalInput")
    w0x = nc.dram_tensor("w0x", [128, KC_IN * NM * 128], BF16, kind="ExternalInput")
    w0h = nc.dram_tensor("w0h", [128, KC_H * NM * 128], BF16, kind="ExternalInput")
    w1x = nc.dram_tensor("w1x", [128, KC_H * NM * 128], BF16, kind="ExternalInput")
    w1h = nc.dram_tensor("w1h", [128, KC_H * NM * 128], BF16, kind="ExternalInput")
    bias0 = nc.dram_tensor("bias0", [128, NM], F32, kind="ExternalInput")
    bias1 = nc.dram_tensor("bias1", [128, NM], F32, kind="ExternalInput")
    y = nc.dram_tensor("y", [T, 128, B], BF16, kind="ExternalOutput")
    hf = nc.dram_tensor("hf", [2, 128, B], F32, kind="ExternalOutput")
    cf = nc.dram_tensor("cf", [2, 128, B], F32, kind="ExternalOutput")

    rg = [list(range(NC_CORES))]

    with tile.TileContext(nc) as tc:
        with (
            tc.tile_pool(name="wp", bufs=1) as wp,
            tc.tile_pool(name="sb", bufs=2) as sb,
            tc.tile_pool(name="xp", bufs=4) as xp,
            tc.tile_pool(name="pp", bufs=1, space="PSUM") as pp,
            tc.tile_pool(name="dr", bufs=2, space="DRAM") as dr,
        ):
            # ---- load weights/biases once ----
            w0x_sb = wp.tile([128, KC_IN * NM * 128], BF16, name="w0x_sb")
            w0h_sb = wp.tile([128, KC_H * NM * 128], BF16, name="w0h_sb")
            w1x_sb = wp.tile([128, KC_H * NM * 128], BF16, name="w1x_sb")
            w1h_sb = wp.tile([128, KC_H * NM * 128], BF16, name="w1h_sb")
            b0_sb = wp.tile([128, NM], F32, name="b0_sb")
            b1_sb = wp.tile([128, NM], F32, name="b1_sb")
            nc.sync.dma_start(w0x_sb[:], w0x[:])
            nc.sync.dma_start(w0h_sb[:], w0h[:])
            nc.sync.dma_start(w1x_sb[:], w1x[:])
            nc.sync.dma_start(w1h_sb[:], w1h[:])
            nc.sync.dma_start(b0_sb[:], bias0[:])
            nc.sync.dma_start(b1_sb[:], bias1[:])
            bias_sb = [b0_sb, b1_sb]

            # ---- initial state ----
            c_prev = [None, None]
            for L in range(2):
                ct = sb.tile([128, B], F32, tag=f"c{L}", name=f"c{L}_init")
                nc.vector.memset(ct[:], 0.0)
                c_prev[L] = ct
            hg_prev = [None, None]  # gathered h.T [128, KC_H, B] per layer

            def wtile(w_sb, c, m):
                return w_sb[:, (c * NM + m) * 128:(c * NM + m + 1) * 128]

            def mm_group(G, parts):
                """parts: (w_sb, rhs_fn, kc) accumulated into the NM PSUM banks."""
                total = sum(kc for (_, _, kc) in parts)
                n = 0
                for w_sb, rhs_fn, kc in parts:
                    for c in range(kc):
                        for m in range(NM):
                            nc.tensor.matmul(
                                G[m][:],
                                lhsT=wtile(w_sb, c, m),
                                rhs=rhs_fn(c),
                                start=(n == 0),
                                stop=(n == total - 1),
                            )
                        n += 1

            def chain(L, t, G):
                """LSTM cell elementwise on gate PSUM banks -> h.T bf16 tile."""
                bs = bias_sb[L]
                sf = sb.tile([128, B], F32, tag=f"sf{L}", name=f"sf{L}_{t}")
                si = sb.tile([128, B], F32, tag=f"si{L}", name=f"si{L}_{t}")
                so = sb.tile([128, B], F32, tag=f"so{L}", name=f"so{L}_{t}")
                tg = sb.tile([128, B], F32, tag=f"tg{L}", name=f"tg{L}_{t}")
                # f first so the DVE can start c*f early
                nc.scalar.activation(sf[:], G[1][:], SIG, bias=bs[:, 1:2])
                nc.scalar.activation(si[:], G[0][:], SIG, bias=bs[:, 0:1])
                nc.scalar.activation(tg[:], G[3][:], TANH, bias=bs[:, 3:4])
                nc.scalar.activation(so[:], G[2][:], SIG, bias=bs[:, 2:3])
                t1 = sb.tile([128, B], F32, tag=f"t1{L}", name=f"t1{L}_{t}")
                t2 = sb.tile([128, B], F32, tag=f"t2{L}", name=f"t2{L}_{t}")
                cn = sb.tile([128, B], F32, tag=f"c{L}", name=f"c{L}_{t}")
                nc.vector.tensor_mul(t1[:], sf[:], c_prev[L][:])
                nc.vector.tensor_mul(t2[:], si[:], tg[:])
                nc.vector.tensor_add(cn[:], t1[:], t2[:])
                c_prev[L] = cn
                tch = sb.tile([128, B], F32, tag=f"tc{L}", name=f"tc{L}_{t}")
                nc.scalar.activation(tch[:], cn[:], TANH)
                hb = sb.tile([128, B], BF16, tag=f"h{L}", name=f"h{L}_{t}")
                nc.vector.tensor_mul(hb[:], so[:], tch[:])
                return hb

            def ag_send(L, t, hb):
                bounce = dr.tile([128, B], BF16, tag=f"bo{L}", name=f"bo{L}_{t}")
                nc.sync.dma_start(bounce[:], hb[:])
                gath = dr.tile([NC_CORES * 128, B], BF16, addr_space="Shared",
                               tag=f"ga{L}", name=f"ga{L}_{t}", bufs=3)
                nc.gpsimd.collective_compute(
                    "AllGather", mybir.AluOpType.bypass, replica_groups=rg,
                    ins=[bounce[:].opt()], outs=[gath[:].opt()],
                )
                return gath

            def ag_recv(L, t, gath):
                half = KC_H // 2
                hga = sb.tile([128, half, B], BF16, tag=f"hga{L}", name=f"hga{L}_{t}")
                hgb = sb.tile([128, half, B], BF16, tag=f"hgb{L}", name=f"hgb{L}_{t}")
                g3 = gath.rearrange("(c p) b -> p c b", p=128)
                nc.sync.dma_start(hga[:], g3[:, :half, :])
                nc.sync.dma_start(hgb[:], g3[:, half:, :])
                hg_prev[L] = (hga, hgb)

            def layer1_mms(tm):
                G1 = [pp.tile([128, B], F32, tag=f"G1m{m}", name=f"G1m{m}_{tm}")
                      for m in range(NM)]
                h0g = hg_prev[0]
                parts = [(w1x_sb, lambda c, _h=h0g: _h[c // 4][:, c % 4, :], KC_H)]
                if tm > 0:
                    h1g = hg_prev[1]
                    parts.append((w1h_sb, lambda c, _h=h1g: _h[c // 4][:, c % 4, :], KC_H))
                mm_group(G1, parts)
                return G1

            # Emission order per block t (engines in brackets):
            #   xt(t) [sync]
            #   L0x(t), L0h(t) [PE]; chain0(t) [ACT/DVE]
            #   bo0(t) [sync], TRIG0(t) [gpsimd]
            #   hg1(t-2) [sync]      <- AG1(t-2) result, consumed by L1h(t-1)
            #   L1x(t-1), L1h(t-1) [PE]; chain1(t-1); y(t-1) [sync]
            #   bo1(t-1) [sync], TRIG1(t-1) [gpsimd]
            #   hg0(t) [sync]        <- AG0(t) result, consumed by L0h(t+1)/L1x(t)
            h1b = None
            last_h0 = None
            ga1_pend = {}
            for t in range(T):
                xt_tile = xp.tile([128, KC_IN * B], BF16, tag="xt", name=f"xt_{t}")
                nc.sync.dma_start(xt_tile[:], xT[t])
                G0 = [pp.tile([128, B], F32, tag=f"G0m{m}", name=f"G0m{m}_{t}")
                      for m in range(NM)]
                parts = [(w0x_sb, lambda c, _x=xt_tile: _x[:, c * B:(c + 1) * B],
                          KC_IN)]
                if t > 0:
                    h0g = hg_prev[0]
                    parts.append((w0h_sb, lambda c, _h=h0g: _h[c // 4][:, c % 4, :], KC_H))
                mm_group(G0, parts)
                h0b = chain(0, t, G0)
                last_h0 = h0b
                ga0 = ag_send(0, t, h0b)

                if t - 2 in ga1_pend:
                    ag_recv(1, t - 2, ga1_pend.pop(t - 2))

                if t > 0:
                    tm = t - 1
                    G1 = layer1_mms(tm)
                    h1b = chain(1, tm, G1)
                    if tm < T - 1:
                        ga1_pend[tm] = ag_send(1, tm, h1b)
                    nc.sync.dma_start(y[tm], h1b[:])

                ag_recv(0, t, ga0)

            # ---------- layer 1, final step ----------
            tm = T - 1
            if tm - 1 in ga1_pend:
                ag_recv(1, tm - 1, ga1_pend.pop(tm - 1))
            G1 = layer1_mms(tm)
            h1b = chain(1, tm, G1)
            nc.sync.dma_start(y[tm], h1b[:])

            # ---------- final h/c ----------
            h0f = sb.tile([128, B], F32, tag="h0f", name="h0f")
            h1f = sb.tile([128, B], F32, tag="h1f", name="h1f")
            nc.vector.tensor_copy(h0f[:], last_h0[:])
            nc.vector.tensor_copy(h1f[:], h1b[:])
            nc.sync.dma_start(hf[0], h0f[:])
            nc.sync.dma_start(hf[1], h1f[:])
            nc.sync.dma_start(cf[0], c_prev[0][:])
            nc.sync.dma_start(cf[1], c_prev[1][:])

    nc.compile()
    _NC_CACHE[T] = nc
    return nc


def _pack_w(W_eff, k):
    """W_eff [4H, D] -> per-core lhsT pack [128, (D/128)*NM*128] bf16.

    pack[p, (c*NM+m)*128 + col] = W_eff[GATE_BLOCKS[m]*H + k*128 + col, c*128 + p]
    """
    D = W_eff.shape[1]
    kc = D // 128
    rows = np.concatenate(
        [bm * H + k * 128 + np.arange(128) for bm in GATE_BLOCKS])
    arr = W_eff[rows].reshape(NM, 128, kc, 128)      # [m, col, c, p]
    arr = arr.transpose(3, 2, 0, 1).reshape(128, kc * NM * 128)
    return np.ascontiguousarray(arr.astype(ml_dtypes.bfloat16))


def _pack_bias(b_eff, k):
    cols = [b_eff[bm * H + k * 128:bm * H + k * 128 + 128] for bm in GATE_BLOCKS]
    return np.ascontiguousarray(np.stack(cols, axis=1).astype(np.float32))


def prep_in_maps(x, w_ih0, b_ih0, a_ih0, w_hh0, b_hh0, a_hh0,
                 w_ih1, b_ih1, a_ih1, w_hh1, b_hh1, a_hh1):
    x = np.asarray(x, dtype=np.float32)
    Bx, T, INx = x.shape
    assert (Bx, INx) == (B, IN)
    W0x = np.asarray(w_ih0, np.float32) * np.float32(a_ih0)
    W0h = np.asarray(w_hh0, np.float32) * np.float32(a_hh0)
    W1x = np.asarray(w_ih1, np.float32) * np.float32(a_ih1)
    W1h = np.asarray(w_hh1, np.float32) * np.float32(a_hh1)
    bias0 = np.asarray(b_ih0, np.float32) + np.asarray(b_hh0, np.float32)
    bias1 = np.asarray(b_ih1, np.float32) + np.asarray(b_hh1, np.float32)

    # xT[t, p, c*B + b] = x[b, t, c*128 + p]
    xT = x.reshape(B, T, KC_IN, 128).transpose(1, 3, 2, 0).reshape(
        T, 128, KC_IN * B)
    xT = np.ascontiguousarray(xT.astype(ml_dtypes.bfloat16))

    in_maps = []
    for k in range(NC_CORES):
        in_maps.append({
            "xT": xT,
            "w0x": _pack_w(W0x, k),
            "w0h": _pack_w(W0h, k),
            "w1x": _pack_w(W1x, k),
            "w1h": _pack_w(W1h, k),
            "bias0": _pack_bias(bias0, k),
            "bias1": _pack_bias(bias1, k),
        })
    return in_maps, T


def assemble_outputs(results, T):
    ys = np.stack([np.asarray(r["y"], dtype=np.float32) for r in results], axis=1)
    out = ys.transpose(3, 0, 1, 2).reshape(B, T, H)
    hfs = np.stack([np.asarray(r["hf"], dtype=np.float32) for r in results], axis=1)
    h_final = hfs.transpose(0, 3, 1, 2).reshape(2, B, H)
    cfs = np.stack([np.asarray(r["cf"], dtype=np.float32) for r in results], axis=1)
    c_final = cfs.transpose(0, 3, 1, 2).reshape(2, B, H)
    return out, h_final, c_final


def kernel(x, w_ih0, b_ih0, a_ih0, w_hh0, b_hh0, a_hh0,
           w_ih1, b_ih1, a_ih1, w_hh1, b_hh1, a_hh1):
    in_maps, T = prep_in_maps(
        x, w_ih0, b_ih0, a_ih0, w_hh0, b_hh0, a_hh0,
        w_ih1, b_ih1, a_ih1, w_hh1, b_hh1, a_hh1)
    nc = build_nc(T)
    res = run_bass_kernel_spmd(nc, in_maps, list(range(NC_CORES)))
    out, h_final, c_final = assemble_outputs(res.results, T)
    return (np.asarray(out, np.float32), np.asarray(h_final, np.float32),
            np.asarray(c_final, np.float32))
